# revision 2
# baseline (speedup 1.0000x reference)
"""GAT layer kernel for Trainium2, SPMD across 8 NeuronCores — v2.

Math (per batch b):
    h[n]   = x[b,n] @ proj_w[n] + proj_b[n]
    s[i,j] = h[i] . a_src[j] + h[j] . a_dst[j]
    att    = softmax_j( mask(leaky_relu(s)) ),  mask: (0<dist<0.5)|eye
    y[i]   = sum_j att[i,j] h[j]

v2 strategy: NO collectives. Each core receives the full projection
inputs and redundantly projects all 4096 nodes (~tens of us of extra
PE time), then computes scores/softmax/aggregation for its own 512
destination rows. Cores never synchronize with each other, so the
per-core NEFF window contains no cross-core wait (the v1 AllGather
made every core absorb the full inter-core dispatch skew).

Per-core j-axis rotation: core k's inputs are rolled by -k*512 along
the node axis so that each core's OWN rows are always j-tiles 0..3.
The SPMD program is identical on every core; softmax/aggregation are
invariant to j permutation.

Layouts (host-packed, bf16):
  wq  [65, 2048, 128]  wq[c,p,e*64+o] = proj_w[2p+e][c,o]; row 64 = bias
  xq  [65, 2048,   8]  xq[c,p,b*2+e]  = x[b,2p+e,c];       row 64 = 1
  adP [128, 32, 4, 64] a_dst per node (128-partition tiles, b-replicated)
  asT [64, 4096]       a_src^T
  neP [128, 32, 512]   1.0 where NOT edge (j on partitions, i free)
  out [4, 512, 64] f32

The projection packs two nodes per matmul (K=65 contraction: 64
channels + bias row; M=128: both nodes' 64 outputs side by side; the
off-block quadrants of the output are garbage and simply not read).
The per-j dst-dot d_j enters the scores as a K=1 matmul (d row @ ones)
so phase D group g only depends on phase A group <= 2g+1, letting the
Tile scheduler overlap projection with the score/softmax pipeline.
"""

import numpy as np
import ml_dtypes

BF16 = ml_dtypes.bfloat16

B = 4
N = 4096
C = 64
C1 = C + 1
R = 8            # cores
NB = N // R      # 512 rows per core
JT = 128         # j-tile width
NJT = N // JT    # 32 j-tiles
GJ = 2           # j-tiles per score/elementwise group
NG = NJT // GJ   # 16 groups
NP2 = N // 2     # node pairs (2048)
ALPHA = 0.01
NEG = -1.0e15

_CACHE = {}


def _build():
    import concourse.bass as bass
    import concourse.tile as tile
    from concourse import bacc, mybir
    from concourse.masks import make_identity

    f32 = mybir.dt.float32
    bf16 = mybir.dt.bfloat16
    Alu = mybir.AluOpType
    Act = mybir.ActivationFunctionType

    nc = bacc.Bacc("TRN2", target_bir_lowering=False, debug=False, num_devices=R)

    wq = nc.dram_tensor("wq", [C1, NP2 * 2 * C], bf16, kind="ExternalInput").ap()
    xq = nc.dram_tensor("xq", [C1, NP2 * 2 * B], bf16, kind="ExternalInput").ap()
    adP = nc.dram_tensor("adP", [JT, NJT * B * C], bf16, kind="ExternalInput").ap()
    asT = nc.dram_tensor("asT", [C, N], bf16, kind="ExternalInput").ap()
    neP = nc.dram_tensor("neP", [JT, NJT * NB], bf16, kind="ExternalInput").ap()
    out = nc.dram_tensor("out", [B, NB, C], f32, kind="ExternalOutput").ap()
    d_dram = nc.dram_tensor("d_dram", [128, 128], bf16, kind="Internal")

    with tile.TileContext(nc) as tc:
        _body(tc, nc, bass, mybir, make_identity, f32, bf16, Alu, Act,
              wq, xq, adP, asT, neP, out, d_dram)
    nc.compile()
    return nc


def _body(tc, nc, bass, mybir, make_identity, f32, bf16, Alu, Act,
          wq, xq, adP, asT, neP, out, d_dram):
    from contextlib import ExitStack
    Axis = mybir.AxisListType

    ctx = ExitStack()
    with ctx:
        const = ctx.enter_context(tc.tile_pool(name="const", bufs=1))
        wpool = ctx.enter_context(tc.tile_pool(name="wpool", bufs=2))
        hpool = ctx.enter_context(tc.tile_pool(name="hpool", bufs=2))
        dpool = ctx.enter_context(tc.tile_pool(name="dpool", bufs=2))
        ewp = ctx.enter_context(tc.tile_pool(name="ewp", bufs=4))
        epp = ctx.enter_context(tc.tile_pool(name="epp", bufs=4))
        psA = ctx.enter_context(tc.tile_pool(name="psA", bufs=1, space="PSUM"))
        psT = ctx.enter_context(tc.tile_pool(name="psT", bufs=1, space="PSUM"))
        psS = ctx.enter_context(tc.tile_pool(name="psS", bufs=2, space="PSUM"))
        psY = ctx.enter_context(tc.tile_pool(name="psY", bufs=2, space="PSUM"))

        # ---- constants ----
        identf = const.tile([128, 128], f32)
        make_identity(nc, identf[:])
        identb = const.tile([128, 128], bf16)
        nc.vector.tensor_copy(identb[:], identf[:])
        iNeg = const.tile([128, 128], bf16)
        # iNeg = NEG * I
        nc.vector.tensor_scalar(iNeg[:], identb[:], -NEG, -1.0,
                                Alu.mult, Alu.mult)
        # ---- resident inputs (issued from Pool's SWDGE: SP queue is the
        # busiest DMA path, Pool is otherwise idle) ----
        xq_res = const.tile([C1, NP2 * 2 * B], bf16)
        nc.gpsimd.dma_start(out=xq_res[:], in_=xq[:, :])
        adP_res = const.tile([JT, NJT * B * C], bf16)
        nc.gpsimd.dma_start(out=adP_res[:], in_=adP[:, :])
        neP_res = const.tile([JT, NJT * NB], bf16)
        qn = NJT * NB // 4
        for s in range(4):
            nc.gpsimd.dma_start(out=neP_res[:, s * qn:(s + 1) * qn],
                                in_=neP[:, s * qn:(s + 1) * qn])

        # ---- resident intermediates ----
        v_all = const.tile([JT, NJT, B, C1], bf16)   # h natural + ones col
        nc.vector.memset(v_all[:, :, :, C:C1], 1.0)
        hT_res = const.tile([C1, B, NB], bf16)       # own rows ^T + ones row
        nc.vector.memset(hT_res[C:C1, :, :], 1.0)
        d_all = const.tile([128, 128], f32)          # d, col = g*4+b
        # waug: score lhsT per b = [a_src^T ; d row]
        waug = const.tile([C1, B, N], bf16)
        for b in range(B):
            nc.gpsimd.dma_start(out=waug[0:C, b, :], in_=asT[:, :])

        # ---------------- phase A: project a group of 128 nodes ----------
        GW = C * 2 * C                            # weight cols per group
        def phase_a(g):
            wgt = wpool.tile([C1, GW], bf16, tag="wg", name="wgt")
            nc.sync.dma_start(out=wgt[:], in_=wq[:, g * GW:(g + 1) * GW])
            ph = psA.tile([128, C, 2 * B], f32, tag="ph")
            for t in range(C):                    # node pairs (2t, 2t+1)
                p = g * C + t
                nc.tensor.matmul(ph[:, t, :],
                                 wgt[:, t * 2 * C:(t + 1) * 2 * C],
                                 xq_res[:, p * 2 * B:(p + 1) * 2 * B],
                                 start=True, stop=True)
            # ph[e*64+o, t, b*2+e] = h[b, 128g+2t+e, o]
            hT_g = hpool.tile([C, B, JT], bf16, tag="hg")
            for e in range(2):
                nc.vector.tensor_copy(
                    hT_g[:, :, e::2],
                    ph[e * C:(e + 1) * C, :, e::2].rearrange("p t b -> p b t"))
            if g < 4:
                nc.vector.tensor_copy(hT_res[0:C, :, g * JT:(g + 1) * JT],
                                      hT_g[:])
            # natural-layout h (+ d dot) via PE transpose
            pt = psT.tile([128, B, C], bf16, tag="pt")
            for b in range(B):
                nc.tensor.transpose(pt[:, b, :], hT_g[:, b, :],
                                    identb[0:C, 0:C])
            nc.scalar.activation(v_all[:, g, :, 0:C], pt[:], Act.Copy)
            # d dot reads v_all (SBUF) so it can live on the idle Pool
            # engine (GPSIMD cannot access PSUM)
            dtmp = dpool.tile([128, B, C], f32, tag="dt")
            nc.gpsimd.tensor_mul(
                dtmp[:], v_all[:, g, :, 0:C],
                adP_res[:, g * B * C:(g + 1) * B * C].rearrange(
                    "p (b c) -> p b c", b=B))
            for b in range(B):
                nc.vector.reduce_sum(d_all[:, g * B + b:g * B + b + 1],
                                     dtmp[:, b, :], axis=Axis.X)

        # d columns for 16 A-groups -> d rows of waug. Matmul output bases
        # must be 0/32/64, so transpose in half chunks, bounce through DRAM
        # to cross partitions, and gather each b's rows (stride 4) into the
        # single-partition d row of waug.
        def d_chunk(c):
            ptd = psT.tile([64, 128], f32, tag="pt", name="ptd")
            nc.tensor.transpose(ptd[:], d_all[:, c * 64:(c + 1) * 64],
                                identf[:])
            dT_tmp = hpool.tile([64, 128], bf16, tag="dtt", name="dT_tmp")
            nc.vector.tensor_copy(dT_tmp[:], ptd[:])
            nc.sync.dma_start(out=d_dram.ap()[c * 64:(c + 1) * 64, :],
                              in_=dT_tmp[:])
            half = 16 * JT
            for b in range(B):
                nc.sync.dma_start(
                    out=waug[C:C1, b, c * half:(c + 1) * half].rearrange(
                        "o (r m) -> o r m", r=16),
                    in_=d_dram.ap()[c * 64 + b:(c + 1) * 64:B, :].rearrange(
                        "(o r) m -> o r m", o=1))

        # ---------------- phase D: one score/softmax/agg group ------------
        psy_map = {}

        def phase_d(b, g):
            if g == 0:
                psy_map[b] = psY.tile([C1, NB], f32, tag="psy", name="psy")
            psy = psy_map[b]
            pss = psS.tile([128, GJ * NB], f32, tag="pss")
            for q in range(GJ):
                jt = g * GJ + q
                sl = pss[:, q * NB:(q + 1) * NB]
                nc.tensor.matmul(sl, waug[:, b, jt * JT:(jt + 1) * JT],
                                 hT_res[:, b, :], start=True, stop=False)
                nc.tensor.matmul(sl, iNeg[:],
                                 neP_res[:, jt * NB:(jt + 1) * NB],
                                 start=False, stop=True)
            # stage scores to bf16 SBUF (only one PSUM input allowed per
            # DVE op); the staging copy alternates ACT/DVE for balance
            tcp = ewp.tile([128, GJ * NB], bf16, tag="tcp")
            if g % 5 == 0:
                nc.vector.tensor_copy(tcp[:], pss[:])
            else:
                nc.scalar.activation(tcp[:], pss[:], Act.Copy)
            u = ewp.tile([128, GJ * NB], bf16, tag="u")
            nc.vector.scalar_tensor_tensor(out=u[:], in0=tcp[:], scalar=ALPHA,
                                           in1=tcp[:], op0=Alu.mult,
                                           op1=Alu.max)
            p = ewp.tile([128, GJ * NB], bf16, tag="p")
            nc.scalar.activation(p[:], u[:], Act.Exp)
            for q in range(GJ):
                jt = g * GJ + q
                nc.tensor.matmul(psy[:], v_all[:, jt, b, :],
                                 p[:, q * NB:(q + 1) * NB],
                                 start=(jt == 0), stop=(jt == NJT - 1))

        def epilogue(b):
            psy = psy_map.pop(b)
            ysb = epp.tile([C1, NB], f32, tag="ysb")
            nc.vector.tensor_copy(ysb[:], psy[:])
            for g4 in range(4):
                pe_ = psT.tile([128, C1], f32, tag="pt", name="pte")
                nc.tensor.transpose(pe_[:], ysb[:, g4 * 128:(g4 + 1) * 128],
                                    identf[0:C1, 0:C1])
                rec = epp.tile([128, 1], f32, tag="rec")
                nc.vector.reciprocal(rec[:], pe_[:, C:C1])
                yo = epp.tile([128, C], f32, tag="yo")
                nc.vector.tensor_scalar(yo[:], pe_[:, 0:C], rec[:], None,
                                        Alu.mult)
                nc.sync.dma_start(out=out[b, g4 * 128:(g4 + 1) * 128, :],
                                  in_=yo[:])

        # ---------------- emission: interleave A with D(b=0,1) ------------
        for g in range(16):         # own rows are j-tiles 0..3
            phase_a(g)
        d_chunk(0)                  # d rows for j-tiles 0..15
        for i in range(8):          # A(16..31) vs D(b0/b1, 0..7)
            phase_a(16 + 2 * i)
            phase_a(17 + 2 * i)
            phase_d(0, i)
            phase_d(1, i)
        d_chunk(1)                  # d rows for j-tiles 16..31
        for g in range(8, NG):
            phase_d(0, g)
            phase_d(1, g)
        epilogue(0)
        epilogue(1)
        for g in range(NG):
            phase_d(2, g)
            phase_d(3, g)
        epilogue(2)
        epilogue(3)


def _get_nc():
    if "nc" not in _CACHE:
        _CACHE["nc"] = _build()
    return _CACHE["nc"]


def _make_in_maps(inputs):
    x = np.asarray(inputs["x"], dtype=np.float32)
    dist_mat = np.asarray(inputs["dist_mat"], dtype=np.float32)
    proj_w = np.asarray(inputs["proj_w"], dtype=np.float32)
    proj_b = np.asarray(inputs["proj_b"], dtype=np.float32)
    a_w = np.asarray(inputs["a_w"], dtype=np.float32)

    # wq/xq base (unrotated), bf16
    wq = np.empty((C1, NP2, 2 * C), dtype=BF16)
    wq[:C] = (proj_w.reshape(NP2, 2, C, C).transpose(2, 0, 1, 3)
              .reshape(C, NP2, 2 * C)).astype(BF16)
    wq[C] = proj_b.reshape(NP2, 2 * C).astype(BF16)
    xqa = np.empty((C1, NP2, 2 * B), dtype=BF16)
    xqa[:C] = (x.reshape(B, NP2, 2, C).transpose(3, 1, 0, 2)
               .reshape(C, NP2, 2 * B)).astype(BF16)
    xqa[C] = np.ones((NP2, 2 * B), dtype=BF16)

    asTb = np.ascontiguousarray(a_w[:, :C].T).astype(BF16)   # [64, 4096]
    adb = a_w[:, C:].astype(BF16)                            # [4096, 64]

    # adjacency -> notE (f32 0/1), with forced diagonal edge
    adj = (dist_mat > 0.0) & (dist_mat < 0.5)
    np.fill_diagonal(adj, True)
    notE = (~adj).astype(BF16)                               # [i, j]

    in_maps = []
    for k in range(R):
        r = k * NB
        wq_k = np.ascontiguousarray(np.roll(wq, -r // 2, axis=1)).reshape(
            C1, -1)
        xq_k = np.ascontiguousarray(np.roll(xqa, -r // 2, axis=1)).reshape(
            C1, -1)
        asT_k = np.ascontiguousarray(np.roll(asTb, -r, axis=1))
        ad_k = np.roll(adb, -r, axis=0)                      # [4096, 64]
        adP_k = np.ascontiguousarray(
            np.broadcast_to(
                ad_k.reshape(NJT, JT, C).transpose(1, 0, 2)[:, :, None, :],
                (JT, NJT, B, C))).reshape(JT, -1)
        ne_k = np.roll(notE[r:r + NB, :], -r, axis=1)        # [512, 4096]
        neP_k = np.ascontiguousarray(
            ne_k.T.reshape(NJT, JT, NB).transpose(1, 0, 2)).reshape(JT, -1)
        in_maps.append({
            "wq": wq_k, "xq": xq_k, "adP": adP_k, "asT": asT_k, "neP": neP_k,
        })
    return in_maps


def kernel(x, dist_mat, proj_w, proj_b, a_w):
    from concourse.bass_utils import run_bass_kernel_spmd

    nc = _get_nc()
    in_maps = _make_in_maps({"x": x, "dist_mat": dist_mat, "proj_w": proj_w,
                             "proj_b": proj_b, "a_w": a_w})
    last_err = None
    for _attempt in range(3):
        try:
            res = run_bass_kernel_spmd(nc, in_maps, core_ids=list(range(R)))
            outs = [res.results[k]["out"] for k in range(R)]
            return np.concatenate(outs, axis=1).astype(np.float32)
        except Exception as e:  # transient runtime/device errors: retry
            last_err = e
    raise last_err
